# revision 39
# baseline (speedup 1.0000x reference)
"""Trainium2 Bass kernel for nn_EngramModule (embedding_lookup).

Sharding: 8 cores; core c handles batch c//2, sequence half c%2 (4096 output
tokens per core). Each core computes 4224 striped positions: local position
ell = 33*p + j (p = SBUF partition, j = column), covering seq range
[s0-2, s0-2+4224) — a 2-token left halo for the causal conv plus tail padding.

End-to-end wall clock over the axon tunnel is transfer-bound (the device
kernel itself is a few tens of ms), so every stage minimizes bytes moved and
keeps transfers concurrent:
  - n-gram hashing runs on host in exact int64 numpy; only the final gather
    indices ship (i16, 16-partition wrap, x8-replicated on device).
  - hidden ships int8 (dynamic scale s = 127/absmax; 1/s ships per-partition
    and is folded into the gate's rsqrt factor on device), split into two
    tensors per core so quantize->upload pipelines at half-core granularity.
  - wpack (fused embedding table + key/value projections + conv weights, all
    f16) ships sharded 1/8th per core and is AllGather'd on device.
  - output is written int8 with per-token f16 scales (absmax over channels via
    ACT Square + DVE top-8 max), fetched with concurrent per-shard requests
    and dequantized on host as each shard lands.
Everything runs through one AOT-compiled shard_map jit; the donated output
buffers are the previous call's output arrays (the kernel writes every
element, so stale contents are harmless).

Device pipeline per core:
  1. AllGather wpack -> wfull; load wk/wv/cw tiles; fused embedding table
     [8192, 128] f16 gathered TRANSPOSED via dma_gather(transpose=True)
     -> memT per head [96(+pad), 4224].
  2. fp16 matmuls (K=96 per head, 8-chunk PSUM accumulation) for key/value
     projections; rmsnorm via ACT Square+accum; gate dot via DVE
     scalar_tensor_tensor accum; sigmoid/sqrt on ACT.
  3. causal depthwise conv along j (free dim) with a partition-shift halo;
     per-token int8 quantization of the conv output.
"""

import sys
import numpy as np

sys.path.insert(0, "/opt/trn_rl_repo")

from contextlib import ExitStack

import concourse.bass as bass
import concourse.bacc as bacc
import concourse.tile as tile
from concourse import mybir
from concourse import bass2jax

F32 = mybir.dt.float32
F16 = mybir.dt.float16
I32 = mybir.dt.int32
I16 = mybir.dt.int16
I8 = mybir.dt.int8
AOT = mybir.AluOpType
AFT = mybir.ActivationFunctionType

# --- problem constants (mirrors reference.py) ---
LAYER_ID = 0
HASH_SEED = 17
N_GRAM_LIST = [2, 3]
NUM_HEADS = 4
HASH_MODULUS = 1023
HIDDEN = 768
HEAD_DIM = 96
CONV_K = 3
EPS = 1e-6
B, S = 4, 8192
VOCAB = 10240

# --- sharding/layout constants ---
NC = 8           # cores
P = 128          # partitions
TB = 33          # tokens per partition (columns)
TC = P * TB      # 4224 computed positions per core
TOUT = 4096      # output tokens per core
NSLOT = 8        # 4 heads x 2 n-grams

# wpack layout in f16 elements over a [WROWS, 128] tensor
FEMB_OFF = 0
FEMB_N = NSLOT * 1024 * P            # 1048576
WK_OFF = FEMB_OFF + FEMB_N
WK_N = HEAD_DIM * NSLOT * HIDDEN     # 589824
WV_OFF = WK_OFF + WK_N
CW_OFF = WV_OFF + WK_N               # 2228224
CW_N = CONV_K * HIDDEN               # 2304
WPACK_N = ((CW_OFF + CW_N + 1023) // 1024) * 1024   # 2231296
WROWS = WPACK_N // P                  # 17432 (divisible by 8)

# per-core f16 blob layout: [wpack shard | w16 bits (16 partitions) | invs]
# hidden ships separately as a per-core int8 tensor (quantized host-side with
# a dynamic scale s = 127/absmax; 1/s rides in the invs slot and is folded
# into the gate's rsqrt factor on device)
WSH_N = WPACK_N // NC                 # 278912
W16_N = NSLOT * 16 * (TC // 16)       # 33792 (16 partitions; replicated x8 on device)
WSH_OFF = 0
BW16_OFF = WSH_N
BINV_OFF = WSH_N + W16_N
BLOB_N = WSH_N + W16_N + P            # 312832
HID_N = TC * HIDDEN                   # 3244032 int8


def _hash_params(n):
    max_int = (1 << 31) - 1
    mults, offs = [], []
    for h in range(NUM_HEADS):
        base = HASH_SEED + 10007 * (LAYER_ID + 1) + 1543 * (n + 1) + 8191 * (h + 1)
        row = []
        for pp in range(n):
            v = (base + 32771 * (pp + 1) + 65537 * (h + 1) * (pp + 1)) % max_int
            row.append(v * 2 + 1)
        mults.append(row)
        offs.append((base * 2147483647 + 97 * (n + h + 1)) % max_int)
    return np.array(mults, dtype=np.int64), np.array(offs, dtype=np.int64)


# position helpers: stream n = j*128 + p holds token ell = 33*p + j
_n = np.arange(TC)
_ell_of_n = TB * (_n % P) + _n // P
# output token t = 33*p + jc - 2 -> (p, jc) for the per-token scale lookup
_P_OF_T = (np.arange(TOUT) + 2) // TB
_J_OF_T = (np.arange(TOUT) + 2) % TB


def _host_hash(input_ids):
    """Exact reference n-gram hashes on host. [B, S] int64 -> [B, S, 8] int64,
    with 0 at padded positions (first n-1)."""
    ids = np.asarray(input_ids, dtype=np.int64)
    out = np.zeros((B, S, NSLOT), dtype=np.int64)
    for gi, n in enumerate(N_GRAM_LIST):
        mult, off = _hash_params(n)           # [H, n], [H]
        mix = ids[:, 0:S - n + 1, None] * mult[None, None, :, 0]
        for p in range(1, n):
            mix = np.bitwise_xor(mix, ids[:, p:S - n + 1 + p, None] * mult[None, None, :, p])
        h = (mix + off[None, None, :]) % HASH_MODULUS + 1   # [B, S-n+1, H]
        out[:, n - 1:, gi * 4:(gi + 1) * 4] = h
    return out


def _build_w16(input_ids):
    """Per-core gather-index tensors [NC, NSLOT, 16, TC//16] int16 (the x8
    partition replication dma_gather wants is done on device)."""
    hashes = _host_hash(input_ids)            # [B, S, 8]
    w16 = np.empty((NC, NSLOT, 16, TC // 16), dtype=np.int16)
    slot_base = 1024 * np.arange(NSLOT, dtype=np.int64)[None, :]
    for c in range(NC):
        b, s0 = c // 2, (c % 2) * TOUT
        g = s0 - 2 + _ell_of_n                # [TC] global pos per stream slot
        gc = np.clip(g, 0, S - 1)
        hv = hashes[b, gc, :]                 # [TC, 8]
        hv[(g < 0) | (g >= S)] = 0
        row = hv + slot_base                  # femb row index, < 8192
        # wrap: stream i -> (i % 16, i // 16)
        w16[c] = row.T.reshape(NSLOT, TC // 16, 16).transpose(0, 2, 1)
    return w16


def _build_wpack(emb, w_key, w_value, key_norm_w, value_norm_w, conv_w):
    """[WPACK_N] f16: femb rows 0..8191, then wk, wv, cw regions."""
    pack = np.zeros(WPACK_N, dtype=np.float16)
    femb = pack[FEMB_OFF:FEMB_OFF + FEMB_N].reshape(NSLOT * 1024, P)
    femb[:, :HEAD_DIM] = emb.reshape(NSLOT * 1024, HEAD_DIM).astype(np.float16)
    femb[::1024, :] = 0.0                    # padding_idx rows, exact zeros

    def wprep(w, nw):
        wt = (w * nw[:, None]).T.astype(np.float16)      # [m, o] = w[o, m]*nw[o]
        out = np.empty((HEAD_DIM, NSLOT * HIDDEN), dtype=np.float16)
        for h in range(NSLOT):
            out[:, h * HIDDEN:(h + 1) * HIDDEN] = wt[h * HEAD_DIM:(h + 1) * HEAD_DIM, :]
        return out

    pack[WK_OFF:WK_OFF + WK_N] = wprep(w_key, key_norm_w).ravel()
    pack[WV_OFF:WV_OFF + WK_N] = wprep(w_value, value_norm_w).ravel()
    pack[CW_OFF:CW_OFF + CW_N] = conv_w.T.astype(np.float16).ravel()   # [3, 768]
    return pack


def _build_nc():
    nc = bacc.Bacc("TRN2", target_bir_lowering=False, num_devices=NC)

    blob = nc.dram_tensor("blob", [BLOB_N], F16, kind="ExternalInput")
    # hidden split into partition halves -> 2 concurrent upload streams/core
    hid8a = nc.dram_tensor("hid8a", [HID_N // 2], I8, kind="ExternalInput")
    hid8b = nc.dram_tensor("hid8b", [HID_N // 2], I8, kind="ExternalInput")
    wstage = nc.dram_tensor("wstage", [WSH_N // P, P], F16)
    wfull = nc.dram_tensor("wfull", [WROWS, P], F16, addr_space="Shared")
    out_d = nc.dram_tensor("out", [TOUT, HIDDEN], I8, kind="ExternalOutput")
    out_s = nc.dram_tensor("outs", [P, TB], F16, kind="ExternalOutput")

    with tile.TileContext(nc) as tc:
        with ExitStack() as ctx:
            _emit(ctx, tc, nc, blob, (hid8a, hid8b), wstage, wfull, out_d, out_s)
    nc.compile()
    return nc


def _emit(ctx, tc, nc, blob, hid8, wstage, wfull, out_d, out_s):
    # hid8 is a pair of DRAM tensors covering partitions [0,64) and [64,128)
    consts = ctx.enter_context(tc.tile_pool(name="consts", bufs=1))
    work = ctx.enter_context(tc.tile_pool(name="work", bufs=2))
    small = ctx.enter_context(tc.tile_pool(name="small", bufs=4))
    gpool = ctx.enter_context(tc.tile_pool(name="gpool", bufs=6))
    psk = ctx.enter_context(tc.tile_pool(name="psk", bufs=1, space="PSUM"))
    psv = ctx.enter_context(tc.tile_pool(name="psv", bufs=3, space="PSUM"))

    def blob_ap(offset, ap):
        return bass.AP(tensor=blob, offset=offset, ap=ap)

    def wf_ap(offset, ap):
        return bass.AP(tensor=wfull, offset=offset, ap=ap)

    # collectives cannot read IO tensors: bounce the wpack shard through an
    # internal DRAM staging tensor, then AllGather to the full pack
    nc.sync.dma_start(out=wstage[:], in_=blob_ap(WSH_OFF, [[P, WSH_N // P], [1, P]]))
    nc.gpsimd.collective_compute(
        kind="AllGather", op=AOT.bypass,
        replica_groups=[list(range(NC))],
        ins=[wstage[:]], outs=[wfull[:]])

    # ---- constants into SBUF ----
    wk_sb = consts.tile([HEAD_DIM, NSLOT * HIDDEN], F16, tag="wk")
    nc.sync.dma_start(out=wk_sb[:], in_=wf_ap(WK_OFF, [[NSLOT * HIDDEN, HEAD_DIM], [1, NSLOT * HIDDEN]]))
    wv_sb = consts.tile([HEAD_DIM, NSLOT * HIDDEN], F16, tag="wv")
    nc.sync.dma_start(out=wv_sb[:], in_=wf_ap(WV_OFF, [[NSLOT * HIDDEN, HEAD_DIM], [1, NSLOT * HIDDEN]]))
    cw16 = consts.tile([P, CW_N], F16, tag="cw16")
    nc.sync.dma_start(out=cw16[:], in_=wf_ap(CW_OFF, [[0, P], [1, CW_N]]))
    cwb = consts.tile([P, CW_N], F32, tag="cwb")
    nc.vector.tensor_copy(out=cwb[:], in_=cw16[:])

    # invs = 1/s for the int8 hidden dequant, folded into the gate scale
    inv16 = consts.tile([P, 1], F16, tag="inv16")
    nc.sync.dma_start(out=inv16[:], in_=blob_ap(BINV_OFF, [[1, P], [1, 1]]))
    invs_t = consts.tile([P, 1], F32, tag="invs")
    nc.vector.tensor_copy(out=invs_t[:], in_=inv16[:])

    w16t = []
    for h in range(NSLOT):
        t = consts.tile([P, TC // 16], I16, tag=f"w16_{h}")
        src = blob_ap(BW16_OFF + h * 16 * (TC // 16),
                      [[TC // 16, 16], [1, TC // 16]]).bitcast(I16)
        nc.sync.dma_start(out=t[0:16, :], in_=src)
        for blk in (16, 32, 64):
            nc.sync.dma_start(out=t[blk:2 * blk, :], in_=t[0:blk, :])
        w16t.append(t)

    # ---- transposed fp16 embedding gathers ----
    memp = ctx.enter_context(tc.tile_pool(name="memp", bufs=1))
    femb_ap = wf_ap(FEMB_OFF, [[P, NSLOT * 1024], [1, P]])
    memT = []
    for h in range(NSLOT):
        m = memp.tile([P, TC], F16, tag=f"memT{h}")
        nc.gpsimd.dma_gather(
            out_ap=m[:].rearrange("p (a b) -> p a b", b=TC),
            in_ap=femb_ap, idxs_ap=w16t[h][:],
            num_idxs=TC, num_idxs_reg=TC, elem_size=P, transpose=True,
            single_packet=False)
        memT.append(m)

    # ---- column loop ----
    # gcols[m] holds gated values at ell = 33p + m - 2. m<4 pinned (late conv
    # cols 0/1 + halo); m>=4 rolling 6-slot window.
    gcols = {}
    for m in range(4):
        gcols[m] = consts.tile([P, HIDDEN], F32, tag=f"gcpin{m}", name=f"gcpin{m}")
    nc.vector.memset(gcols[0][:], 0.0)
    nc.vector.memset(gcols[1][:], 0.0)

    def value_col(j):
        if j + 2 >= 4:
            gcols[j + 2] = gpool.tile([P, HIDDEN], F32, tag="gcroll", name="gcroll")
        hid8_j = work.tile([P, HIDDEN], I8, tag="hid8")
        nc.sync.dma_start(
            out=hid8_j[0:P // 2, :],
            in_=bass.AP(tensor=hid8[0], offset=j * HIDDEN,
                        ap=[[TB * HIDDEN, P // 2], [1, HIDDEN]]))
        nc.sync.dma_start(
            out=hid8_j[P // 2:P, :],
            in_=bass.AP(tensor=hid8[1], offset=j * HIDDEN,
                        ap=[[TB * HIDDEN, P // 2], [1, HIDDEN]]))
        hid_j = work.tile([P, HIDDEN], F16, tag="hid")
        nc.vector.tensor_copy(out=hid_j[:], in_=hid8_j[:])
        pk = psk.tile([P, HIDDEN], F32, tag="pk")
        pv = psv.tile([P, HIDDEN], F32, tag="pv")
        for ps, wsb in ((pk, wk_sb), (pv, wv_sb)):
            for h in range(NSLOT):
                lhs = memT[h][0:HEAD_DIM, j * P:(j + 1) * P]
                nc.tensor.matmul(out=ps[:, 0:512],
                                 lhsT=lhs, rhs=wsb[:, h * HIDDEN: h * HIDDEN + 512],
                                 start=(h == 0), stop=(h == NSLOT - 1))
                nc.tensor.matmul(out=ps[:, 512:HIDDEN],
                                 lhsT=lhs, rhs=wsb[:, h * HIDDEN + 512:(h + 1) * HIDDEN],
                                 start=(h == 0), stop=(h == NSLOT - 1))
        scr = work.tile([P, HIDDEN], F32, tag="scr")
        ssq_k = small.tile([P, 1], F32, tag="ssqk")
        nc.scalar.activation(out=scr[:], in_=pk[:], func=AFT.Square, accum_out=ssq_k[:])
        scr2 = work.tile([P, HIDDEN], F32, tag="scr2")
        dot = small.tile([P, 1], F32, tag="dot")
        nc.vector.scalar_tensor_tensor(
            out=scr2[:], in0=hid_j[:], scalar=1.0, in1=pk[:],
            op0=AOT.mult, op1=AOT.mult, accum_out=dot[:])
        scr3 = work.tile([P, HIDDEN], F32, tag="scr3")
        ssq_v = small.tile([P, 1], F32, tag="ssqv")
        nc.scalar.activation(out=scr3[:], in_=pv[:], func=AFT.Square, accum_out=ssq_v[:])

        rk = small.tile([P, 1], F32, tag="rk")
        nc.vector.tensor_scalar_add(rk[:], ssq_k[:], float(HIDDEN) * EPS)
        nc.vector.reciprocal(rk[:], rk[:])
        nc.scalar.activation(out=rk[:], in_=rk[:], func=AFT.Sqrt)
        nc.vector.tensor_mul(rk[:], rk[:], invs_t[:])   # dequant: dot is s*true
        gate = small.tile([P, 1], F32, tag="gate")
        nc.scalar.activation(out=gate[:], in_=dot[:], func=AFT.Sigmoid, scale=rk[:])
        rv = small.tile([P, 1], F32, tag="rv")
        nc.vector.tensor_scalar_add(rv[:], ssq_v[:], float(HIDDEN) * EPS)
        nc.vector.reciprocal(rv[:], rv[:])
        nc.scalar.activation(out=rv[:], in_=rv[:], func=AFT.Sqrt, scale=float(HIDDEN))
        gv = small.tile([P, 1], F32, tag="gv")
        nc.vector.tensor_mul(gv[:], gate[:], rv[:])
        nc.scalar.activation(out=gcols[j + 2][:], in_=pv[:], func=AFT.Copy, scale=gv[:])

    scales_t = consts.tile([P, TB], F16, tag="scales")

    def conv_col(jc):
        a = work.tile([P, HIDDEN], F32, tag="cva")
        b = work.tile([P, HIDDEN], F32, tag="cvb")
        c = work.tile([P, HIDDEN], F32, tag="cvc")
        nc.vector.tensor_mul(a[:], gcols[jc][:], cwb[:, 0:HIDDEN])
        nc.vector.tensor_mul(b[:], gcols[jc + 1][:], cwb[:, HIDDEN:2 * HIDDEN])
        nc.vector.tensor_mul(c[:], gcols[jc + 2][:], cwb[:, 2 * HIDDEN:3 * HIDDEN])
        nc.gpsimd.tensor_add(a[:], a[:], b[:])
        nc.gpsimd.tensor_add(a[:], a[:], c[:])
        # per-token (per-partition-row) int8 quantization: scale = 127/absmax
        sq = work.tile([P, HIDDEN], F32, tag="cvsq")
        nc.scalar.activation(out=sq[:], in_=a[:], func=AFT.Square)
        m8 = small.tile([P, 8], F32, tag="m8")
        nc.vector.max(m8[:], sq[:])
        inv = small.tile([P, 1], F32, tag="qinv")
        nc.scalar.activation(out=inv[:], in_=m8[:, 0:1], func=AFT.Sqrt,
                             scale=1.0 / (127.0 * 127.0))
        nc.vector.tensor_scalar_add(inv[:], inv[:], 1e-12)
        nc.vector.tensor_copy(out=scales_t[:, jc:jc + 1], in_=inv[:])
        qs = small.tile([P, 1], F32, tag="qs")
        nc.vector.reciprocal(qs[:], inv[:])
        q = work.tile([P, HIDDEN], I8, tag="cvq")
        nc.scalar.activation(out=q[:], in_=a[:], func=AFT.Copy, scale=qs[:])
        p0 = 1 if jc < 2 else 0
        pmax = (4095 - (jc - 2)) // TB
        np_rows = pmax - p0 + 1
        dst = bass.AP(tensor=out_d, offset=(TB * p0 + jc - 2) * HIDDEN,
                      ap=[[TB * HIDDEN, np_rows], [1, HIDDEN]])
        nc.sync.dma_start(out=dst, in_=q[p0:pmax + 1, :])

    for j in range(TB):
        value_col(j)
        if j >= 4:
            conv_col(j - 2)   # jc 2..30; 0/1 need the partition halo below
    # halo columns from partition p-1's last two value columns
    nc.sync.dma_start(out=gcols[0][1:P, :], in_=gcols[TB][0:P - 1, :])
    nc.sync.dma_start(out=gcols[1][1:P, :], in_=gcols[TB + 1][0:P - 1, :])
    conv_col(TB - 2)
    conv_col(TB - 1)
    conv_col(0)
    conv_col(1)
    nc.sync.dma_start(out=out_s[:], in_=scales_t[:])


_STATE = None


def _get_state():
    global _STATE
    if _STATE is not None:
        return _STATE
    import jax
    import jax.numpy as jnp
    from jax.sharding import Mesh, PartitionSpec, NamedSharding
    from jax.experimental.shard_map import shard_map

    nc = _build_nc()
    bass2jax.install_neuronx_cc_hook()
    assert nc.dbg_addr is None or not nc.dbg_callbacks

    partition_name = nc.partition_id_tensor.name if nc.partition_id_tensor else None
    in_names, out_names, out_avals = [], [], []
    for alloc in nc.m.functions[0].allocations:
        if not isinstance(alloc, mybir.MemoryLocationSet):
            continue
        name = alloc.memorylocations[0].name
        if alloc.kind == "ExternalInput":
            if name != partition_name:
                in_names.append(name)
        elif alloc.kind == "ExternalOutput":
            out_names.append(name)
            out_avals.append(jax.core.ShapedArray(
                tuple(alloc.tensor_shape), mybir.dt.np(alloc.dtype)))
    assert in_names == ["blob", "hid8a", "hid8b"], in_names
    n_params = len(in_names)
    n_outs = len(out_avals)
    in_names_full = list(in_names) + out_names
    if partition_name is not None:
        in_names_full.append(partition_name)
    donate = tuple(range(n_params, n_params + n_outs))

    def _body(*args):
        operands = list(args)
        if partition_name is not None:
            operands.append(bass2jax.partition_id_tensor())
        outs = bass2jax._bass_exec_p.bind(
            *operands,
            out_avals=tuple(out_avals),
            in_names=tuple(in_names_full),
            out_names=tuple(out_names),
            lowering_input_output_aliases=(),
            sim_require_finite=True,
            sim_require_nnan=True,
            nc=nc,
        )
        return tuple(outs)

    devices = jax.devices()[:NC]
    assert len(devices) == NC
    mesh = Mesh(np.asarray(devices), ("core",))
    sh_core = NamedSharding(mesh, PartitionSpec("core"))
    in_specs = (PartitionSpec("core"),) * (n_params + n_outs)
    out_specs = (PartitionSpec("core"),) * n_outs
    sharded = jax.jit(
        shard_map(_body, mesh=mesh, in_specs=in_specs, out_specs=out_specs,
                  check_rep=False),
        donate_argnums=donate, keep_unused=True)

    examples = [np.zeros((NC * BLOB_N,), np.float16),
                np.zeros((NC * HID_N // 2,), np.int8),
                np.zeros((NC * HID_N // 2,), np.int8)]
    zero_outs = [np.zeros((NC * a.shape[0], *a.shape[1:]), a.dtype) for a in out_avals]
    compiled = sharded.lower(*examples, *zero_outs).compile()

    zeros_fn = jax.jit(
        lambda: tuple(jnp.zeros((NC * a.shape[0], *a.shape[1:]), a.dtype)
                      for a in out_avals),
        out_shardings=(sh_core,) * n_outs)

    _STATE = dict(jax=jax, sh_core=sh_core, compiled=compiled, zeros_fn=zeros_fn,
                  devices=devices, next_out=None)
    return _STATE


def kernel(hidden_states, input_ids, emb, w_key, w_value, key_norm_w,
           value_norm_w, conv_w):
    st = _get_state()
    jax = st["jax"]

    import concurrent.futures as cf
    hidden_states = np.asarray(hidden_states, dtype=np.float32)
    # dynamic int8 quantization of hidden: s = 127/absmax, no clipping.
    # quantize per core in threads and enqueue each device transfer as soon
    # as that core's shard is ready.
    ex = cf.ThreadPoolExecutor(16)
    hflat = hidden_states.reshape(16, -1)
    maxes = list(ex.map(lambda i: np.abs(hflat[i]).max(), range(16)))
    qscale = np.float32(127.0) / max(max(maxes), 1e-30)
    hq = np.empty((NC, TC, HIDDEN), dtype=np.int8)
    HTC = TC // 2    # rows per partition-half (ell range per hid8a/b tensor)

    def _quant_put(c, half_p):
        # quantize hid rows [half_p*HTC, (half_p+1)*HTC) of core c and ship
        b, s0 = c // 2, (c % 2) * TOUT
        r0 = half_p * HTC
        g0 = s0 - 2 + r0                      # global pos of first row
        lo = max(0, -g0)
        hi = min(HTC, S - g0)
        sl = hq[c, r0:r0 + HTC]
        if lo > 0:
            sl[:lo] = 0
        if hi < HTC:
            sl[hi:] = 0
        sl[lo:hi] = np.rint(
            hidden_states[b, g0 + lo: g0 + hi] * qscale).astype(np.int8)
        return jax.device_put(sl.reshape(HID_N // 2), st["devices"][c])

    hid_futs = [[ex.submit(_quant_put, c, hp) for c in range(NC)]
                for hp in range(2)]

    blob = np.empty((NC, BLOB_N), dtype=np.float16)
    w16 = _build_w16(input_ids)               # [NC, NSLOT, 16, 264] i16
    blob[:, BW16_OFF:BW16_OFF + W16_N] = w16.reshape(NC, -1).view(np.float16)
    blob[:, BINV_OFF:BINV_OFF + P] = np.float16(1.0 / qscale)
    pack = _build_wpack(np.asarray(emb, dtype=np.float32),
                        np.asarray(w_key, dtype=np.float32),
                        np.asarray(w_value, dtype=np.float32),
                        np.asarray(key_norm_w, dtype=np.float32),
                        np.asarray(value_norm_w, dtype=np.float32),
                        np.asarray(conv_w, dtype=np.float32))
    blob[:, WSH_OFF:WSH_OFF + WSH_N] = pack.reshape(NC, WSH_N)
    blob_dev = jax.device_put(blob.reshape(NC * BLOB_N), st["sh_core"])
    hid_devs = [jax.make_array_from_single_device_arrays(
        (NC * HID_N // 2,), st["sh_core"], [f.result() for f in hid_futs[hp]])
        for hp in range(2)]

    # dispatch + fetch, with one retry on transient device failure
    out = np.empty((B, S, HIDDEN), dtype=np.float32)
    try:
        for attempt in range(2):
            donate = st["next_out"] if st["next_out"] is not None else st["zeros_fn"]()
            st["next_out"] = None
            try:
                outs = st["compiled"](blob_dev, hid_devs[0], hid_devs[1], *donate)
                # concurrent shard fetches (the tunnel pipelines outstanding
                # requests); dequantize into the final f32 output as each
                # shard lands
                shards = sorted(outs[0].addressable_shards,
                                key=lambda s: s.index[0].start or 0)
                for s in shards:
                    s.data.copy_to_host_async()
                sfut = ex.submit(lambda: np.asarray(outs[1]))
                futs = [ex.submit(lambda s=s: np.asarray(s.data)) for s in shards]
                s_all = sfut.result().reshape(NC, P, TB)
                for c, f in enumerate(futs):
                    b, half = c // 2, c % 2
                    sl = out[b, half * TOUT:(half + 1) * TOUT]
                    sl[...] = f.result()
                    sl *= s_all[c][_P_OF_T, _J_OF_T].astype(np.float32)[:, None]
                st["next_out"] = tuple(outs)
                return out
            except Exception:
                if attempt == 1:
                    raise
    finally:
        ex.shutdown(wait=False)
    return out
